# revision 30
# baseline (speedup 1.0000x reference)
"""Trainium2 Bass kernel for EditOuterAttention (dense transformer cross-attention).

Reference computation (BS=2, LX=LY=2048, D=1024, H=16, DK=64):
    q = x @ Wq + bq ; k = y @ Wk + bk ; v = y @ Wv + bv     (per batch)
    scores = q @ k^T / sqrt(DK) + mask
    out = (softmax(scores) @ v) @ Wo + bo

Sharding: 8 cores = 2 (batch) x 4 (head groups of 4 heads / 256 dims).
Per core (batch b, head-group g):
    - column-parallel QKV projections over the 256-dim head slice
    - attention for 4 heads
    - row-parallel output projection -> partial O^T [1024, 2048] (bf16)
    - partials are summed on the HOST (no on-device collective; the
      serialized ReduceScatter tail cost ~75us in an earlier version).

Dataflow notes:
    - All matmuls run in bf16 with fp32 PSUM accumulation (an fp8
      DoubleRow AV variant measured 2x faster on the PE but its ~2% output
      noise exceeds the error budget: attention outputs are random-sign
      sums, so weight noise passes through at full strength).
    - Q^T/K^T are produced in [head_dim, seq] layout by using the weight
      matrix as the stationary operand (out = W^T @ x^T).
    - Scores are computed transposed (S^T[sy, sx]) so the exp'd tiles feed
      the AV matmul as the moving operand with no transpose.
    - The softmax denominator comes for free from the ones-column of the
      [V | 1] stationary operand.  Normalization: DVE copy of the PSUM
      denominator row to SBUF (reciprocal_approx_fast misreads PSUM
      partition-64 sources on HW), DVE reciprocal_approx_fast, bf16 cast,
      a K=1 bf16 PE matmul broadcast across the 64 head-dim partitions
      (deferred one block so the PE never waits on the DVE chain), DVE
      multiply into AO.
    - Emission interleaves scores / exp / AV per 2-sy-tile step plus a
      filler queue (V projection drains inside block 0, remaining Q/K
      projection tiles, O-projection tiles, normalize phase B) to keep the
      PE queue dense: the tensor engine must stay continuously busy >3us
      to hold its 2.4GHz p-state.
    - Input DMA is issued in exact first-consumption order (wq, xT quarter
      0, wk/yT quarter 0 interleaved, remaining yT quarters, wv, ...), so
      the first scores matmul fires ~6us in and the Exp pipeline (the
      critical engine) starts early.
    - 1/sqrt(DK) is folded into the Exp activation's scale; zero biases
      and zero mask (the common case) compile out entirely.  Nonzero
      bq/bk are applied on-chip, nonzero bv/bo are exact host-side
      post-corrections, and a nonzero mask multiplies exp(mask)^T into
      the exp'd score tiles.
"""

from collections import deque

import numpy as np
import ml_dtypes

import concourse.bass as bass
import concourse.bacc as bacc
import concourse.tile as tile
import concourse.mybir as mybir
from concourse.bass_utils import run_bass_kernel_spmd

BS, LX, LY, D, H, DK = 2, 2048, 2048, 1024, 16, 64
NCORES = 8
NGRP = 4            # head groups (tensor-parallel)
HD = H * DK // NGRP  # 256 head dims per core
NH = H // NGRP       # 4 heads per core
SXB = 512            # sx block
NSXB = LX // SXB     # 4
NSYT = LY // 128     # 16 sy tiles
NPAIR = NSYT // 2    # 8 sy-tile pairs (DoubleRow contraction)
VHALF = 272          # fp8 V-pair half stride (>= NH*65, 16B aligned, even)
NDC = D // 128       # 8 contraction chunks
NET = D // 128       # 8 output-feature tiles

F32 = mybir.dt.float32
BF16 = mybir.dt.bfloat16
F8 = mybir.dt.float8e4
EXPF = mybir.ActivationFunctionType.Exp
DR = mybir.MatmulPerfMode.DoubleRow

_compiled = {}


def _build(has_qk_bias: bool, has_mask: bool, n_cores: int = NCORES,
           with_collective: bool = True):
    nc = bacc.Bacc("TRN2", target_bir_lowering=False, debug=False,
                   num_devices=n_cores)

    xT = nc.dram_tensor("xT", [D, LX], BF16, kind="ExternalInput")
    yT = nc.dram_tensor("yT", [D, LY], BF16, kind="ExternalInput")
    wq = nc.dram_tensor("wq", [D, HD], BF16, kind="ExternalInput")
    wk = nc.dram_tensor("wk", [D, HD], BF16, kind="ExternalInput")
    wv = nc.dram_tensor("wv", [D, HD], BF16, kind="ExternalInput")
    wo = nc.dram_tensor("wo", [HD, D], BF16, kind="ExternalInput")
    if has_qk_bias:
        bq = nc.dram_tensor("bq", [HD], F32, kind="ExternalInput")
        bk = nc.dram_tensor("bk", [HD], F32, kind="ExternalInput")
    if has_mask:
        em = nc.dram_tensor("em", [LY, LX], BF16, kind="ExternalInput")
    out_ext = nc.dram_tensor("out", [D, LX], BF16, kind="ExternalOutput")

    with tile.TileContext(nc) as tc:
        with (
            tc.tile_pool(name="persist", bufs=1) as pp,
            tc.tile_pool(name="st", bufs=3) as stp,
            tc.tile_pool(name="ostage", bufs=3) as osp,
            tc.tile_pool(name="small", bufs=4) as smp,
            # PSUM budget (8 banks): scores 2x[128,1024] = 4, AV accum
            # 2x[65,512] = 2, projection staging 1, normalize broadcast 1.
            tc.tile_pool(name="scp", bufs=2, space="PSUM") as scp,
            tc.tile_pool(name="avp", bufs=2, space="PSUM") as avp,
            tc.tile_pool(name="mmp", bufs=1, space="PSUM") as mmp,
            tc.tile_pool(name="pbp", bufs=1, space="PSUM") as pbp,
        ):
            # ---- static inputs -> SBUF, in first-consumption order ----
            wq_sb = pp.tile([128, NDC * HD], BF16, tag="wq")
            wk_sb = pp.tile([128, NDC * HD], BF16, tag="wk")
            wv_sb = pp.tile([128, NDC * HD], BF16, tag="wv")
            xT_sb = pp.tile([128, NDC * LX], BF16, tag="xT")
            yT_sb = pp.tile([128, NDC * LY], BF16, tag="yT")

            # Each dma_start costs ~565ns of serial SP-sequencer issue
            # time, so chunk-pairs are merged into single multi-dim-AP
            # transfers and issued in exact first-consumption order.
            def dma_w2(dst_sb, src, d):
                # weight chunks d, d+1 in one issue
                nc.sync.dma_start(
                    out=dst_sb[:, d * HD:(d + 2) * HD]
                        .rearrange("p (c f) -> p c f", f=HD),
                    in_=src.ap().rearrange("(c p) f -> p c f", p=128)
                        [:, d:d + 2, :])

            def dma_seq2(dst_sb, src, d, q0, nq):
                # seq chunks d, d+1, quarters q0..q0+nq-1 in one issue
                sview = src.ap().rearrange("(c p) n -> p c n", p=128)
                dview = dst_sb[:].rearrange("p (c n) -> p c n", n=LY)
                nc.sync.dma_start(
                    out=dview[:, d:d + 2, q0 * SXB:(q0 + nq) * SXB],
                    in_=sview[:, d:d + 2, q0 * SXB:(q0 + nq) * SXB])

            for d in range(0, NDC, 2):    # Q proj t0 sb0 is first compute
                dma_w2(wq_sb, wq, d)
                dma_seq2(xT_sb, xT, d, 0, 1)
            if has_qk_bias:
                bq_sb = pp.tile([128, 2], F32, tag="bq")
                bk_sb = pp.tile([128, 2], F32, tag="bk")
                nc.sync.dma_start(out=bq_sb[:], in_=bq.ap().rearrange("(t p) -> p t", p=128))
                nc.sync.dma_start(out=bk_sb[:], in_=bk.ap().rearrange("(t p) -> p t", p=128))
            for d in range(0, NDC, 2):    # K proj t0, quarter 0 first
                dma_w2(wk_sb, wk, d)
                dma_seq2(yT_sb, yT, d, 0, 1)
            for d in range(0, NDC, 2):    # rest of yT (K t0 sb1-3, V, K t1)
                dma_seq2(yT_sb, yT, d, 1, 3)
            for d in range(0, NDC, 2):    # V proj weights (block-1 fillers)
                dma_w2(wv_sb, wv, d)
            for d in range(0, NDC, 2):    # rest of xT (Q fillers)
                dma_seq2(xT_sb, xT, d, 1, 3)
            wo_sb = pp.tile([128, 2 * D], BF16, tag="wo")
            nc.sync.dma_start(
                out=wo_sb[:].rearrange("p (c f) -> p c f", f=D),
                in_=wo.ap().rearrange("(c p) f -> p c f", p=128))

            QT_sb = pp.tile([128, 2 * LX], BF16, tag="QT")
            KT_sb = pp.tile([128, 2 * LY], BF16, tag="KT")
            V1_sb = pp.tile([128, NSYT * NH * 65], BF16, tag="V1")
            AO_sb = pp.tile([128, 2 * LX], BF16, tag="AO")
            ones_bf = pp.tile([1, 64], BF16, tag="ones")
            nc.vector.memset(ones_bf[:], 1.0)
            ones_cols = V1_sb[:].rearrange("p (t h c) -> p t h c",
                                           t=NSYT, c=65)[:, :, :, 64:65]
            nc.vector.memset(ones_cols, 1.0)

            # ---- emission units (generators yield every ~2 matmuls) ----
            # mmp and pbp are single-bank pools; units alternate between
            # them where possible so back-to-back units stay
            # double-buffered (pbp doubles as the normalize-broadcast bank
            # from block 2 onwards).
            def qk_unit(w_sb, src_sb, dst_sb, b_sb, t, sb, pool=None):
                # 128-dim slice t of Q^T/K^T for sx/sy block sb:
                # dst[hd 128, seq 512] = W_slice^T @ x^T_block
                ps = (pool or mmp).tile([128, SXB], F32, tag="mm", name="psqk")
                for d in range(NDC):
                    nc.tensor.matmul(
                        ps[:],
                        lhsT=w_sb[:, d * HD + t * 128: d * HD + (t + 1) * 128],
                        rhs=src_sb[:, d * LX + sb * SXB: d * LX + sb * SXB + SXB],
                        start=(d == 0), stop=(d == NDC - 1))
                    if d in (1, 3, 5):
                        yield
                dst = dst_sb[:, t * LX + sb * SXB: t * LX + sb * SXB + SXB]
                if b_sb is not None:
                    nc.vector.tensor_scalar_add(dst, ps[:], b_sb[:, t:t + 1])
                else:
                    nc.vector.tensor_copy(dst, ps[:])

            def v_unit(st, pool=None):
                # V rows for sy tile st, all 4 heads -> fp8 [V | 1] half
                # (st%2) of pair st//2
                ps = (pool or mmp).tile([128, SXB], F32, tag="mm", name="psv")
                for d in range(NDC):
                    nc.tensor.matmul(
                        ps[:, 0:HD],
                        lhsT=yT_sb[:, d * LY + st * 128: d * LY + st * 128 + 128],
                        rhs=wv_sb[:, d * HD:(d + 1) * HD],
                        start=(d == 0), stop=(d == NDC - 1))
                    if d in (1, 3, 5):
                        yield
                dst = V1_sb[:, st * NH * 65:(st + 1) * NH * 65] \
                    .rearrange("p (h c) -> p h c", c=65)[:, :, 0:64]
                nc.vector.tensor_copy(dst, ps[:, 0:HD].rearrange("p (h c) -> p h c", c=64))

            def oproj_unit(sb, et):
                po = mmp.tile([128, SXB], F32, tag="mm", name="po")
                for c in range(2):
                    nc.tensor.matmul(
                        po[:],
                        lhsT=wo_sb[:, c * D + et * 128: c * D + (et + 1) * 128],
                        rhs=AO_sb[:, c * LX + sb * SXB: c * LX + sb * SXB + SXB],
                        start=(c == 0), stop=(c == 1))
                    if c == 0:
                        yield
                ost = osp.tile([128, SXB], BF16, tag="ost")
                nc.vector.tensor_copy(ost[:], po[:])
                nc.sync.dma_start(
                    out=out_ext[et * 128:(et + 1) * 128, sb * SXB:(sb + 1) * SXB],
                    in_=ost[:])

            fillers = deque()
            _DONE = object()

            def pop_fillers(n):
                while n and fillers:
                    if next(fillers[0], _DONE) is _DONE:
                        fillers.popleft()
                    else:
                        n -= 1

            def drain(gen):
                for _ in gen:
                    pass

            # ---- mask (rare path): exp(mask)^T blocks per sx block ----
            em_blocks = {}

            def load_mask_block(sb):
                mb = stp.tile([128, NSYT * SXB], BF16, tag="mask", bufs=2)
                for st in range(NSYT):
                    nc.sync.dma_start(
                        out=mb[:, st * SXB:(st + 1) * SXB],
                        in_=em[st * 128:(st + 1) * 128, sb * SXB:(sb + 1) * SXB])
                em_blocks[sb] = mb

            # ---- normalization (split in two phases) ------------------
            # Phase A (right after the AV accumulation stops, all DVE).
            # Phase B (PE broadcast + DVE multiply) goes into the filler
            # queue so the PE reaches it well after the DVE chain drained.
            def norm_a(pav, sb, h):
                un = smp.tile([64, SXB], BF16, tag="un", bufs=4)
                nc.vector.tensor_copy(un[:], pav[0:64, :])
                # reciprocal_approx_fast misreads a PSUM partition-64
                # source on HW (fine in CoreSim) — stage to SBUF first.
                dns = smp.tile([1, SXB], F32, tag="dns")
                nc.vector.tensor_copy(dns[:], pav[64:65, :])
                den = smp.tile([1, SXB], F32, tag="den")
                nc.vector.reciprocal_approx_fast(den[:], dns[:])
                dbf = smp.tile([1, SXB], BF16, tag="dbf")
                nc.vector.tensor_copy(dbf[:], den[:])
                return un, dbf

            def norm_b_unit(un, dbf, sb, h):
                ht, hr = h // 2, (h % 2) * 64
                pbc = pbp.tile([128, SXB], F32, tag="mm", name="pbc")
                nc.tensor.matmul(pbc[0:64, :], lhsT=ones_bf[:], rhs=dbf[:],
                                 start=True, stop=True)
                yield
                dst = AO_sb[hr:hr + 64,
                            ht * LX + sb * SXB: ht * LX + sb * SXB + SXB]
                nc.vector.tensor_mul(dst, un[:], pbc[0:64, :])

            def av_matmul(pav, pST, ph, s2):
                for j in range(2):
                    sy = 2 * s2 + j
                    nc.tensor.matmul(
                        pav[:],
                        lhsT=V1_sb[:, sy * NH * 65 + ph * 65: sy * NH * 65 + ph * 65 + 65],
                        rhs=pST[:, sy * SXB:(sy + 1) * SXB],
                        start=(sy == 0), stop=(sy == NSYT - 1))

            # ---- prefix: Q proj (t0, sb0) only; K proj t0 interleaves
            # with block 0's scores steps, V proj drains via fillers.
            bkb = bk_sb if has_qk_bias else None
            bqb = bq_sb if has_qk_bias else None
            pools = (mmp, pbp)
            drain(qk_unit(wq_sb, xT_sb, QT_sb, bqb, 0, 0, pool=pbp))

            for st in range(NSYT):
                fillers.append(v_unit(st, pool=pools[st % 2]))
            for syb in range(NSXB):
                fillers.append(qk_unit(wk_sb, yT_sb, KT_sb, bkb, 1, syb,
                                       pool=pools[syb % 2]))
            fillers.append(qk_unit(wq_sb, xT_sb, QT_sb, bqb, 1, 0))
            for sb in range(1, NSXB):
                fillers.append(qk_unit(wq_sb, xT_sb, QT_sb, bqb, 0, sb))
                fillers.append(qk_unit(wq_sb, xT_sb, QT_sb, bqb, 1, sb))

            # ---- attention blocks: sx-block-major, heads inner ---------
            blocks = [(sb, h) for sb in range(NSXB) for h in range(NH)]
            prev = None  # (ST tile, sb, h) of the block whose AV is pending

            for bi, (sb, h) in enumerate(blocks):
                if has_mask and h == 0:
                    load_mask_block(sb)
                ht, hr = h // 2, (h % 2) * 64
                ST = stp.tile([128, NSYT * SXB], BF16, tag="st")
                pav = (avp.tile([65, SXB], F32, tag="av", name=f"pav{bi}")
                       if prev else None)
                for s2 in range(NPAIR):
                    if bi == 0 and s2 % 2 == 0:
                        # K proj t0 for sy block s2//2, just ahead of the
                        # scores matmuls that consume it
                        drain(qk_unit(wk_sb, yT_sb, KT_sb, bkb, 0, s2 // 2,
                                      pool=pools[(s2 // 2) % 2]))
                    ps = scp.tile([128, 1024], F32, tag="sc")
                    for j in range(2):
                        sy = 2 * s2 + j
                        nc.tensor.matmul(
                            ps[:, j * SXB:(j + 1) * SXB],
                            lhsT=KT_sb[hr:hr + 64, ht * LY + sy * 128: ht * LY + sy * 128 + 128],
                            rhs=QT_sb[hr:hr + 64, ht * LX + sb * SXB: ht * LX + sb * SXB + SXB],
                            start=True, stop=True)
                    dst = ST[:, s2 * 1024:(s2 + 1) * 1024]
                    nc.scalar.activation(dst, ps[:], EXPF, scale=1.0 / (DK ** 0.5))
                    if has_mask:
                        mb = em_blocks[sb]
                        nc.vector.tensor_mul(dst, dst, mb[:, s2 * 1024:(s2 + 1) * 1024])
                    pop_fillers((0, 11, 4).__getitem__(bi) if bi < 3 else 2)
                    if prev:
                        pST, psb, ph = prev
                        av_matmul(pav, pST, ph, s2)
                if prev:
                    pST, psb, ph = prev
                    un, dbf = norm_a(pav, psb, ph)
                    fillers.append(norm_b_unit(un, dbf, psb, ph))
                    if ph == NH - 1:
                        for et in range(NET):
                            fillers.append(oproj_unit(psb, et))
                prev = (ST, sb, h)

            # ---- tail: AV + normalize of the last block, then O proj ---
            # Drain all but a few filler units before the final normalize so
            # the reserved units cover the DVE reciprocal chain's latency
            # (the PE would otherwise idle ~3us waiting for the broadcast).
            pST, psb, ph = prev
            pav = avp.tile([65, SXB], F32, tag="av", name="pavtail")
            for s2 in range(NPAIR):
                av_matmul(pav, pST, ph, s2)
                if len(fillers) > 3:
                    pop_fillers(2)
            un, dbf = norm_a(pav, psb, ph)
            fillers.append(norm_b_unit(un, dbf, psb, ph))
            for et in range(NET):
                fillers.append(oproj_unit(psb, et))
            while fillers:
                pop_fillers(1000)

    nc.compile()
    return nc


def _get_compiled(has_qk_bias: bool, has_mask: bool):
    key = (has_qk_bias, has_mask)
    if key not in _compiled:
        _compiled[key] = _build(has_qk_bias, has_mask)
    return _compiled[key]


def _prep_inputs(x, y, mask, Wq, bq, Wk, bk, Wv, bv, Wo, bo,
                 has_qk_bias, has_mask):
    bf = ml_dtypes.bfloat16
    xT = [np.ascontiguousarray(x[b].T).astype(bf) for b in range(BS)]
    yT = [np.ascontiguousarray(y[b].T).astype(bf) for b in range(BS)]
    if has_mask:
        em = [np.ascontiguousarray(np.exp(mask[b, 0]).T).astype(bf)
              for b in range(BS)]
    in_maps = []
    for c in range(NCORES):
        b, g = c // NGRP, c % NGRP
        sl = slice(g * HD, (g + 1) * HD)
        m = {
            "xT": xT[b], "yT": yT[b],
            "wq": np.ascontiguousarray(Wq[:, sl]).astype(bf),
            "wk": np.ascontiguousarray(Wk[:, sl]).astype(bf),
            "wv": np.ascontiguousarray(Wv[:, sl]).astype(bf),
            "wo": np.ascontiguousarray(Wo[sl, :]).astype(bf),
        }
        if has_qk_bias:
            m["bq"] = np.ascontiguousarray(bq[sl]).astype(np.float32)
            m["bk"] = np.ascontiguousarray(bk[sl]).astype(np.float32)
        if has_mask:
            m["em"] = em[b]
        in_maps.append(m)
    return in_maps


def kernel(x, y, mask, Wq, bq, Wk, bk, Wv, bv, Wo, bo):
    x = np.asarray(x, np.float32)
    y = np.asarray(y, np.float32)
    mask = np.asarray(mask, np.float32)
    has_qk_bias = bool(np.any(bq) or np.any(bk))
    has_mask = bool(np.any(mask))
    nc = _get_compiled(has_qk_bias, has_mask)
    in_maps = _prep_inputs(x, y, mask, Wq, bq, Wk, bk, Wv, bv, Wo, bo,
                           has_qk_bias, has_mask)
    res = run_bass_kernel_spmd(nc, in_maps, list(range(NCORES)))
    out = np.empty((BS, LX, D), np.float32)
    for b in range(BS):
        # host-side reduction of the 4 head-group partials (row-parallel
        # O projection), then transpose back to [LX, D]
        OT = res.results[b * NGRP]["out"].astype(np.float32)
        for r in range(1, NGRP):
            OT += res.results[b * NGRP + r]["out"].astype(np.float32)
        out[b] = OT.T
    bv = np.asarray(bv, np.float32)
    bo = np.asarray(bo, np.float32)
    if bv.any() or bo.any():
        # softmax rows sum to 1 => v-bias passes through attention exactly.
        out += (bv @ np.asarray(Wo, np.float32) + bo)[None, None, :]
    return out
